# revision 1
# baseline (speedup 1.0000x reference)
"""Spiking transformer block (SpikingRetention + spiking MLP) on 8 Trainium2 cores.

Data-parallel over B=8 (one batch element per NeuronCore), weights replicated.

Layouts: activations are feature-major ("transposed", [C part, N free]) so the
folded BatchNorm is a per-partition ACT affine. LIF membranes are kept
2^t-scaled (A_t = 2^t u_t) so each step is add / compare / masked-reset with
exact power-of-2 constants.

Matmul precision plan (LIF thresholds are sharp -> general matmuls need ~fp32):
  - q/k/v/fc1 (dense x dense): 3-term fp16 split of 4096-scaled operands.
  - proj/fc2 (binary x dense): spikes exact fp16, weights 2-term fp16 split.
  - scores: binary x binary fp16, exact integer accumulation.
  - (S*D)@v: fp32 matmul; S*D produced fp32 by DVE from PSUM.
"""

from contextlib import ExitStack

import numpy as np

import concourse.bacc as bacc
import concourse.tile as tile
from concourse import mybir
from concourse.bass_utils import run_bass_kernel_spmd

f32 = mybir.dt.float32
fp16 = mybir.dt.float16
Alu = mybir.AluOpType
Act = mybir.ActivationFunctionType

T, B, N, C = 4, 8, 512, 512
HID = 2048
H = 8
EPS = 1e-5
NT = N // 128
CT = C // 128
HT = HID // 128

_CACHE = {}


def _lif_step(nc, pool, cpool, ps, t, sc_pow, bias_ap, theta, carry_in, spike_tile, carry_tag):
    """One LIF timestep on a [128,512] tile.
    A_t = carry_in + 2^(t-1) * (2^-sc_pow * psum + bias) ; spike = A>=theta*2^t;
    carry_out = A * (A < theta*2^t). bias_ap is pre-scaled by 2^(t-1)."""
    sc = float(2.0 ** (t - 1 - sc_pow))
    if carry_in is None:
        A = pool.tile([128, 512], f32, name="lifA", tag="lifA", bufs=3)
        if bias_ap is None:
            nc.scalar.activation(A[:], ps[:], Act.Copy, bias=0.0, scale=sc)
        else:
            nc.scalar.activation(A[:], ps[:], Act.Identity, bias=bias_ap, scale=sc)
    else:
        zb = pool.tile([128, 512], f32, name="lifzb", tag="lifzb", bufs=3)
        if bias_ap is None:
            nc.scalar.activation(zb[:], ps[:], Act.Copy, bias=0.0, scale=sc)
        else:
            nc.scalar.activation(zb[:], ps[:], Act.Identity, bias=bias_ap, scale=sc)
        A = pool.tile([128, 512], f32, name="lifA", tag="lifA", bufs=3)
        nc.vector.tensor_tensor(A[:], carry_in[:], zb[:], Alu.add)
    thr = float(theta * (2.0 ** t))
    nc.vector.tensor_scalar(spike_tile[:], A[:], thr, None, Alu.is_ge)
    if carry_tag is not None:
        Cn = cpool.tile([128, 512], f32, name="lifC", tag=carry_tag, bufs=1)
        nc.vector.scalar_tensor_tensor(Cn[:], A[:], thr, A[:], Alu.is_lt, Alu.mult)
        return Cn
    return None


def _build():
    nc = bacc.Bacc("TRN2", target_bir_lowering=False, debug=False)

    xb = nc.declare_dram_parameter("xb", [T, CT, 128, N], f32, isOutput=False)
    ident16_e = nc.declare_dram_parameter("ident16", [128, 128], fp16, isOutput=False)
    w_in = {}
    for nm, ci, co in [("qw", C, C), ("kw", C, C), ("vw", C, C), ("pw", C, C),
                       ("w1", C, HID), ("w2", HID, C)]:
        for part in ("h", "l"):
            w_in[nm + part] = nc.declare_dram_parameter(nm + part, [ci, co], fp16,
                                                        isOutput=False)
    b_in = {}
    for nm, co in [("qb", C), ("kb", C), ("vb", C), ("pb", C), ("b1", HID), ("b2", C)]:
        b_in[nm] = nc.declare_dram_parameter(nm, [128, T, co // 128], f32, isOutput=False)
    dmat_e = nc.declare_dram_parameter("dmat", [H, NT, 128, N], f32, isOutput=False)
    out_e = nc.declare_dram_parameter("out", [T, CT, 128, N], f32, isOutput=True)

    with tile.TileContext(nc) as tc, ExitStack() as ctx:
        pers = ctx.enter_context(tc.tile_pool(name="pers", bufs=1))
        work = ctx.enter_context(tc.tile_pool(name="work", bufs=1))

        ident16 = pers.tile([128, 128], fp16, name="ident16")
        bias_sb = {}

        os_ = {}
        with tc.tile_pool(name="pw_pool", bufs=1) as pw_pool, \
             tc.tile_pool(name="spk_o_pool", bufs=1) as spk_o_pool:
            pwh = pw_pool.tile([128, CT, C], fp16, name="w_pwh")
            pwl = pw_pool.tile([128, CT, C], fp16, name="w_pwl")

            with tc.tile_pool(name="qkvn_pool", bufs=1) as qkvn_pool:
                qs = {}
                ks = {}
                vn = {}
                with tc.tile_pool(name="wqkv_pool", bufs=1) as wqkv_pool, \
                     tc.tile_pool(name="xhl_pool", bufs=1) as xhl_pool, \
                     tc.tile_pool(name="vsT_pool", bufs=1) as vsT_pool, \
                     tc.tile_pool(name="psA", bufs=1, space="PSUM") as psA:
                    # ---- stage X: load pre-transposed x, build scaled fp16 splits
                    xh = {}
                    xl = {}
                    for t in range(T):
                        for ct in range(CT):
                            xt = xhl_pool.tile([128, N], f32, name="xT", tag="xT", bufs=6)
                            nc.sync.dma_start(xt[:], xb[t, ct])
                            xht = xhl_pool.tile([128, N], fp16, name="xh", tag="xh", bufs=16)
                            nc.scalar.activation(xht[:], xt[:], Act.Copy, bias=0.0, scale=4096.0)
                            xh[t, ct] = xht
                            xlt = xhl_pool.tile([128, N], fp16, name="xl", tag="xl", bufs=16)
                            nc.vector.scalar_tensor_tensor(xlt[:], xt[:], 4096.0, xht[:],
                                                           Alu.mult, Alu.subtract)
                            xl[t, ct] = xlt

                    # ---- weights/biases loaded after x so stage X starts immediately
                    wq = {}
                    for nm in ("qw", "kw", "vw"):
                        for part in ("h", "l"):
                            wt = wqkv_pool.tile([128, CT, C], fp16, name=f"w_{nm}{part}")
                            src = w_in[nm + part].rearrange("(kt p) o -> p kt o", p=128)
                            for kt in range(CT):
                                nc.sync.dma_start(wt[:, kt, :], src[:, kt, :])
                            wq[nm + part] = wt
                    nc.sync.dma_start(ident16[:], ident16_e[:, :])
                    for nm, co in [("qb", C), ("kb", C), ("vb", C), ("pb", C),
                                   ("b1", HID), ("b2", C)]:
                        bt = pers.tile([128, T, co // 128], f32, name=f"bs_{nm}")
                        nc.sync.dma_start(bt[:], b_in[nm][:, :, :])
                        bias_sb[nm] = bt
                    nc.sync.dma_start(pwh[:], w_in["pwh"].rearrange("(kt p) o -> p kt o", p=128))
                    nc.sync.dma_start(pwl[:], w_in["pwl"].rearrange("(kt p) o -> p kt o", p=128))

                    # ---- stage QKV
                    vsT = {}
                    for nm, bnm in [("qw", "qb"), ("kw", "kb"), ("vw", "vb")]:
                        for ot in range(CT):
                            pss = []
                            for t in range(T):
                                ps = psA.tile([128, N], f32, name="psq", tag="psq", bufs=6)
                                for kt in range(CT):
                                    lhH = wq[nm + "h"][:, kt, ot * 128:(ot + 1) * 128]
                                    lhL = wq[nm + "l"][:, kt, ot * 128:(ot + 1) * 128]
                                    nc.tensor.matmul(ps[:], lhH, xh[t, kt][:],
                                                     start=(kt == 0), stop=False)
                                    nc.tensor.matmul(ps[:], lhL, xh[t, kt][:],
                                                     start=False, stop=False)
                                    nc.tensor.matmul(ps[:], lhH, xl[t, kt][:],
                                                     start=False, stop=(kt == CT - 1))
                                pss.append(ps)
                            carry = None
                            for t in range(1, T + 1):
                                if nm == "vw":
                                    st = vsT_pool.tile([128, N], fp16, name="spk_v",
                                                       tag="spk_v", bufs=16)
                                    vsT[t - 1, ot] = st
                                else:
                                    st = qkvn_pool.tile([128, N], fp16, name=f"spk_{nm}",
                                                        tag=f"spk_{nm}", bufs=16)
                                    if nm == "qw":
                                        qs[t - 1, ot] = st
                                    else:
                                        ks[t - 1, ot] = st
                                carry = _lif_step(nc, work, xhl_pool, pss[t - 1], t, 24,
                                                  bias_sb[bnm][:, t - 1, ot:ot + 1], 1.0,
                                                  carry, st, f"c_{nm}" if t < T else None)

                    # ---- v spikes -> natural layout via matmul with identity
                    # out[nt-block, c] = vsT[ct-block].T : lhsT = vsT slice, rhs = I16
                    for t in range(T):
                        for nt in range(NT):
                            ps = psA.tile([128, C], f32, name="psv", tag="psx", bufs=2)
                            for ct in range(CT):
                                nc.tensor.matmul(
                                    ps[:, ct * 128:(ct + 1) * 128],
                                    vsT[t, ct][:, nt * 128:(nt + 1) * 128],
                                    ident16[:], start=True, stop=True)
                            vt = qkvn_pool.tile([128, C], f32, name="vn", tag="vn", bufs=16)
                            nc.scalar.copy(vt[:], ps[:])
                            vn[t, nt] = vt

                # ---- stage RET (retention) : head pairs
                with tc.tile_pool(name="dpool", bufs=2) as dpool, \
                     tc.tile_pool(name="spool", bufs=1) as spool, \
                     tc.tile_pool(name="psR", bufs=1, space="PSUM") as psR:
                    for hp in range(H // 2):
                        h0, h1 = 2 * hp, 2 * hp + 1
                        dm0 = dpool.tile([128, NT, N], f32, name="dm0", tag="dm0")
                        nc.sync.dma_start(dm0[:], dmat_e[h0].rearrange("mt p n -> p mt n"))
                        dm1 = dpool.tile([128, NT, N], f32, name="dm1", tag="dm1")
                        nc.sync.dma_start(dm1[:], dmat_e[h1].rearrange("mt p n -> p mt n"))
                        carry = None
                        pso_all = []
                        for t in range(T):
                            stiles = []
                            for mt in range(NT):
                                ps0 = psR.tile([128, N], f32, name="ps_s0", tag="ps_s0", bufs=2)
                                nc.tensor.matmul(ps0[:],
                                                 ks[t, hp][0:64, mt * 128:(mt + 1) * 128],
                                                 qs[t, hp][0:64, :], start=True, stop=True)
                                ps1 = psR.tile([128, N], f32, name="ps_s1", tag="ps_s1", bufs=2)
                                nc.tensor.matmul(ps1[:],
                                                 ks[t, hp][64:128, mt * 128:(mt + 1) * 128],
                                                 qs[t, hp][64:128, :], start=True, stop=True)
                                s0 = spool.tile([128, N], f32, name="sd0", tag="sd0", bufs=3)
                                nc.vector.tensor_tensor(s0[:], ps0[:], dm0[:, mt, :], Alu.mult)
                                s1 = spool.tile([128, N], f32, name="sd1", tag="sd1", bufs=3)
                                nc.vector.tensor_tensor(s1[:], ps1[:], dm1[:, mt, :], Alu.mult)
                                stiles.append((s0, s1))
                            pso = psR.tile([128, N], f32, name="ps_o", tag="ps_o", bufs=4)
                            for mt in range(NT):
                                s0, s1 = stiles[mt]
                                nc.tensor.matmul(pso[0:64, :],
                                                 vn[t, mt][:, h0 * 64:(h0 + 1) * 64], s0[:],
                                                 start=(mt == 0), stop=(mt == NT - 1))
                                nc.tensor.matmul(pso[64:128, :],
                                                 vn[t, mt][:, h1 * 64:(h1 + 1) * 64], s1[:],
                                                 start=(mt == 0), stop=(mt == NT - 1))
                            pso_all.append(pso)
                        for t in range(1, T + 1):
                            st = spk_o_pool.tile([128, N], fp16, name="spk_os",
                                                 tag="spk_os", bufs=16)
                            carry = _lif_step(nc, work, spool, pso_all[t - 1], t, 0, None, 0.5,
                                              carry, st, "c_o" if t < T else None)
                            os_[t - 1, hp] = st

            # ---- merged PROJ + MLP + output, t-outer with carried LIF membranes.
            # wmlp pool opens after qkvn closed; its DMAs overlap retention tail.
            with tc.tile_pool(name="wmlp_pool", bufs=1) as wmlp_pool, \
                 tc.tile_pool(name="mwork", bufs=1) as mwork, \
                 tc.tile_pool(name="xtin_pool", bufs=1) as xtin_pool, \
                 tc.tile_pool(name="psM", bufs=1, space="PSUM") as psM:
                wmlp = {}
                for nm, kt_n, co in [("w1", CT, HID), ("w2", HT, C)]:
                    for part in ("h", "l"):
                        wt = wmlp_pool.tile([128, kt_n, co], fp16, name=f"w_{nm}{part}")
                        nc.sync.dma_start(
                            wt[:], w_in[nm + part].rearrange("(kt p) o -> p kt o", p=128))
                        wmlp[nm + part] = wt

                cp = {}
                c1 = {}
                c2 = {}
                for t in range(1, T + 1):
                    # proj linear + LIF -> attn spikes; x2 = x + attn (SBUF only)
                    x2 = {}
                    x2h = {}
                    x2l = {}
                    for ot in range(CT):
                        ps = psM.tile([128, N], f32, name="psp", tag="psp", bufs=2)
                        for kt in range(CT):
                            nc.tensor.matmul(ps[:], pwh[:, kt, ot * 128:(ot + 1) * 128],
                                             os_[t - 1, kt][:], start=(kt == 0), stop=False)
                            nc.tensor.matmul(ps[:], pwl[:, kt, ot * 128:(ot + 1) * 128],
                                             os_[t - 1, kt][:], start=False,
                                             stop=(kt == CT - 1))
                        stp = mwork.tile([128, N], f32, name="spk_p", tag="spk_p", bufs=2)
                        cp[ot] = _lif_step(nc, work, mwork, ps, t, 12,
                                           bias_sb["pb"][:, t - 1, ot:ot + 1], 1.0,
                                           cp.get(ot), stp, f"cp_{ot}" if t < T else None)
                        xt = xtin_pool.tile([128, N], f32, name="xtin", tag="xtin", bufs=3)
                        nc.sync.dma_start(xt[:], xb[t - 1, ot])
                        x2t = mwork.tile([128, N], f32, name="x2t", tag="x2t", bufs=4)
                        nc.vector.tensor_tensor(x2t[:], xt[:], stp[:], Alu.add)
                        x2[ot] = x2t
                        xht = mwork.tile([128, N], fp16, name="x2h", tag="x2h", bufs=5)
                        nc.scalar.activation(xht[:], x2t[:], Act.Copy, bias=0.0, scale=4096.0)
                        x2h[ot] = xht
                        xlt = mwork.tile([128, N], fp16, name="x2l", tag="x2l", bufs=5)
                        nc.vector.scalar_tensor_tensor(xlt[:], x2t[:], 4096.0, xht[:],
                                                       Alu.mult, Alu.subtract)
                        x2l[ot] = xlt
                    htile = {}
                    for ot in range(HT):
                        ps = psM.tile([128, N], f32, name="psf1", tag="psf1", bufs=3)
                        for kt in range(CT):
                            lhH = wmlp["w1h"][:, kt, ot * 128:(ot + 1) * 128]
                            lhL = wmlp["w1l"][:, kt, ot * 128:(ot + 1) * 128]
                            nc.tensor.matmul(ps[:], lhH, x2h[kt][:], start=(kt == 0),
                                             stop=False)
                            nc.tensor.matmul(ps[:], lhL, x2h[kt][:], start=False, stop=False)
                            nc.tensor.matmul(ps[:], lhH, x2l[kt][:], start=False,
                                             stop=(kt == CT - 1))
                        st = mwork.tile([128, N], fp16, name="spk_h", tag="spk_h",
                                        bufs=HT)
                        c1[ot] = _lif_step(nc, work, mwork, ps, t, 24,
                                           bias_sb["b1"][:, t - 1, ot:ot + 1], 1.0,
                                           c1.get(ot), st, f"c1_{ot}" if t < T else None)
                        htile[ot] = st
                    oT = {}
                    for ot in range(CT):
                        ps = psM.tile([128, N], f32, name="psf2", tag="psf2", bufs=2)
                        for kt in range(HT):
                            nc.tensor.matmul(ps[:],
                                             wmlp["w2h"][:, kt, ot * 128:(ot + 1) * 128],
                                             htile[kt][:], start=(kt == 0), stop=False)
                            nc.tensor.matmul(ps[:],
                                             wmlp["w2l"][:, kt, ot * 128:(ot + 1) * 128],
                                             htile[kt][:], start=False, stop=(kt == HT - 1))
                        st = mwork.tile([128, N], f32, name="spk_m", tag="spk_m", bufs=2)
                        c2[ot] = _lif_step(nc, work, mwork, ps, t, 12,
                                           bias_sb["b2"][:, t - 1, ot:ot + 1], 1.0,
                                           c2.get(ot), st, f"c2_{ot}" if t < T else None)
                        ot_t = mwork.tile([128, N], f32, name="outT", tag="outT", bufs=4)
                        nc.vector.tensor_tensor(ot_t[:], x2[ot][:], st[:], Alu.add)
                        oT[ot] = ot_t
                    for ct in range(CT):
                        nc.sync.dma_start(out_e[t - 1, ct], oT[ct][:])

    nc.finalize()
    return nc


def _host_prep(inputs):
    def fold(w, b, bn):
        g, bb, m, v = [bn[i].astype(np.float64) for i in range(4)]
        A = g / np.sqrt(v + EPS)
        W = w.astype(np.float64) * A[:, None]
        bias = (b.astype(np.float64) - m) * A + bb
        return W, bias

    def split16(WT):
        Ws = WT * 4096.0
        wh = Ws.astype(np.float16)
        wl = (Ws - wh.astype(np.float64)).astype(np.float16)
        return wh, wl

    def bias_layout(bias):
        co = bias.shape[0]
        arr = np.stack([(bias * (2.0 ** t)).reshape(co // 128, 128).T
                        for t in range(T)], axis=1)
        return np.ascontiguousarray(arr.astype(np.float32))

    feed = {}
    bias_key = {"qw": "qb", "kw": "kb", "vw": "vb", "pw": "pb", "w1": "b1", "w2": "b2"}
    for nm, wkey, bkey, bnkey in [("qw", "qw", "qb", "qbn"), ("kw", "kw", "kb", "kbn"),
                                  ("vw", "vw", "vb", "vbn"), ("pw", "pw", "pb", "pbn"),
                                  ("w1", "w1", "b1", "bn1"), ("w2", "w2", "b2", "bn2")]:
        W, bias = fold(inputs[wkey], inputs[bkey], inputs[bnkey])
        wh, wl = split16(np.ascontiguousarray(W.T))
        feed[nm + "h"] = wh
        feed[nm + "l"] = wl
        feed[bias_key[nm]] = bias_layout(bias)

    gamma = 1.0 - 2.0 ** (-5.0 - np.arange(H, dtype=np.float64))
    idx = np.arange(N, dtype=np.float64)
    dist = np.abs(idx[:, None] - idx[None, :])
    scale = (C // H) ** -0.5
    dm = np.empty((H, NT, 128, N), np.float32)
    for h in range(H):
        dm[h] = ((gamma[h] ** dist) * scale).reshape(NT, 128, N).astype(np.float32)
    feed["dmat"] = dm
    feed["ident16"] = np.eye(128, dtype=np.float16)
    return feed


def kernel(**inputs):
    if "nc" not in _CACHE:
        _CACHE["nc"] = _build()
    nc = _CACHE["nc"]
    feed = _host_prep(inputs)
    x = inputs["x"]
    in_maps = []
    for b in range(B):
        m = dict(feed)
        xt = x[:, b].transpose(0, 2, 1).reshape(T, CT, 128, N)
        m["xb"] = np.ascontiguousarray(xt)
        in_maps.append(m)
    res = None
    last_err = None
    for _attempt in range(3):
        try:
            res = run_bass_kernel_spmd(nc, in_maps, list(range(B)))
            break
        except Exception as e:  # transient NRT device wedges recover on retry
            last_err = e
    if res is None:
        raise last_err
    out = np.empty((T, B, N, C), np.float32)
    for b in range(B):
        oT = res.results[b]["out"].reshape(T, C, N)
        out[:, b] = oT.transpose(0, 2, 1)
    return out



# revision 39
# speedup vs baseline: 2.3093x; 2.3093x over previous
"""Spiking transformer block (SpikingRetention + spiking MLP) on 8 Trainium2 cores.

Data-parallel over B=8 (one batch element per NeuronCore), weights replicated.

Activations are feature-major ([C partitions, N free]); folded BatchNorm is a
per-partition affine on the ACT engine. LIF membranes are 2^t-scaled
(A_t = 2^t u_t) so each step is compare / masked-reset with exact power-of-2
constants.

Matmul plan (TRN2 PE cost: cycles = out_free x cycles_per_row):
  - q/k/v, fc1 (dense x dense): single-pass float32r (both operands rounded
    to 12-bit mantissa; 1 cycle/row at free>=256). x is pre-rounded on host.
  - proj, fc2 (binary x dense): fp8e4 DoubleRow (0.5 cycles/row, 2 k-tiles
    per instruction), weights as 2 e4m3 terms with per-row pow2 scales.
  - scores: fp8e4 x fp8e4 spikes, exact integer accumulation.
  - (S*D)@v: fp16 throughout (S exact ints in fp16, D fp16, v binary fp16).

LIF: the carried membrane is folded into the PSUM accumulation with a tiny
diagonal matmul (carry * 2^-(t-1) added by the PE), so each step is just
ACT affine (PSUM->SBUF) + DVE spike (is_ge) + DVE masked-reset carry.
The fp8 layers use per-row scales, so their carry is added on Pool instead.
All S*D products run on the (otherwise idle) Pool/GpSimd engine.
"""

from contextlib import ExitStack

import numpy as np
import ml_dtypes

import concourse.bacc as bacc
import concourse.tile as tile
from concourse import mybir
from concourse.bass_utils import run_bass_kernel_spmd

f32 = mybir.dt.float32
f32r = mybir.dt.float32r
fp16 = mybir.dt.float16
fp8 = mybir.dt.float8e4
Alu = mybir.AluOpType
Act = mybir.ActivationFunctionType
DR = mybir.MatmulPerfMode.DoubleRow

T, B, N, C = 4, 8, 512, 512
HID = 2048
H = 8
EPS = 1e-5
NT = N // 128
CT = C // 128
HT = HID // 128

_CACHE = {}


def _build():
    nc = bacc.Bacc("TRN2", target_bir_lowering=False, debug=False)

    xbr = nc.declare_dram_parameter("xbr", [T, CT, 128, N], f32r, isOutput=False)
    xb = nc.declare_dram_parameter("xb", [T, CT, 128, N], f32, isOutput=False)
    ident16_e = nc.declare_dram_parameter("ident16", [128, 128], fp16, isOutput=False)
    # diag(2^-(t-1)) for folding the LIF carry into PSUM, t = 2..4
    idc_e = nc.declare_dram_parameter("idc", [3, 128, 128], f32r, isOutput=False)
    w_in = {}
    for nm, ci, co in [("qw", C, C), ("kw", C, C), ("vw", C, C), ("w1", C, HID)]:
        w_in[nm] = nc.declare_dram_parameter(nm + "r", [ci, co], f32r, isOutput=False)
    for nm, ci, co in [("pw", C, C), ("w2", HID, C)]:
        for j in range(2):
            w_in[f"{nm}{j}"] = nc.declare_dram_parameter(f"{nm}{j}", [ci, co], fp8,
                                                         isOutput=False)
    consts_e = nc.declare_dram_parameter("consts", [128, T, 44], f32, isOutput=False)
    dmat_e = nc.declare_dram_parameter("dmat", [H, NT, 128, N], fp16, isOutput=False)
    out_e = nc.declare_dram_parameter("out", [T, CT, 128, N], f32, isOutput=True)

    with tile.TileContext(nc) as tc, ExitStack() as ctx:
        pers = ctx.enter_context(tc.tile_pool(name="pers", bufs=1))
        work = ctx.enter_context(tc.tile_pool(name="work", bufs=1))

        ident16 = pers.tile([128, 128], fp16, name="ident16")
        idc = [pers.tile([128, 128], f32r, name=f"idc{t}") for t in (2, 3, 4)]
        bias_sb = {}
        sc_sb = {}

        def carry_mm(ps, t, carry):
            """Fold carry*2^-(t-1) into a PSUM accumulation group (opens it)."""
            nc.tensor.matmul(ps[:], idc[t - 2][:], carry[:], start=True, stop=False)

        def lif_fold(t, ps, bias_ap, theta, spike_ap, carry_tag, cpool):
            """Carry already folded in PSUM. A = 2^(t-1)*ps + bias (ACT);
            spike (DVE); carry' = A*(A<thr) in f32r (DVE)."""
            sc = float(2.0 ** (t - 1))
            A = work.tile([128, N], f32, name="lifA", tag="lifA", bufs=6)
            if bias_ap is None:
                nc.scalar.activation(A[:], ps[:], Act.Copy, bias=0.0, scale=sc)
            else:
                nc.scalar.activation(A[:], ps[:], Act.Identity, bias=bias_ap, scale=sc)
            thr = float(theta * (2.0 ** t))
            nc.vector.tensor_scalar(spike_ap, A[:], thr, None, Alu.is_ge)
            if carry_tag is not None:
                Cn = cpool.tile([128, N], f32r, name="lifC", tag=carry_tag, bufs=1)
                nc.vector.scalar_tensor_tensor(Cn[:], A[:], thr, A[:], Alu.is_lt, Alu.mult)
                return Cn
            return None

        def lif_pool(t, ps, bias_ap, scale_ap, carry_in, spike_ap, carry_tag, cpool):
            """fp8-layer path (per-row scale): zb = scale*ps + bias (ACT);
            A = zb + carry (Pool); spike / carry (DVE)."""
            zb = work.tile([128, N], f32, name="lifzb", tag="lifzb", bufs=4)
            nc.scalar.activation(zb[:], ps[:], Act.Identity, bias=bias_ap, scale=scale_ap)
            if carry_in is None:
                A = zb
            else:
                A = work.tile([128, N], f32, name="lifA", tag="lifA", bufs=6)
                nc.gpsimd.tensor_tensor(A[:], carry_in[:], zb[:], Alu.add)
            thr = float(2.0 ** t)
            nc.vector.tensor_scalar(spike_ap, A[:], thr, None, Alu.is_ge)
            if carry_tag is not None:
                Cn = cpool.tile([128, N], f32, name="lifCp", tag=carry_tag, bufs=1)
                nc.vector.scalar_tensor_tensor(Cn[:], A[:], thr, A[:], Alu.is_lt, Alu.mult)
                return Cn
            return None

        with tc.tile_pool(name="pw8_pool", bufs=1) as pw8_pool, \
             tc.tile_pool(name="spk_o_pool", bufs=1) as spk_o_pool, \
             tc.tile_pool(name="wmlp_pool", bufs=1) as wmlp_pool:
            # o spikes: fp8 pair tiles [128, 2, N]; j-th tile holds head-pairs 2j, 2j+1
            os8 = {}
            for t in range(T):
                for j in range(2):
                    os8[t, j] = spk_o_pool.tile([128, 2, N], fp8, name=f"os8_{t}_{j}")

            with tc.tile_pool(name="qkvn_pool", bufs=1) as qkvn_pool:
                qs = {}
                ks = {}
                vn = {}
                with tc.tile_pool(name="wqkv_pool", bufs=1) as wqkv_pool, \
                     tc.tile_pool(name="xr_pool", bufs=1) as xr_pool, \
                     tc.tile_pool(name="vsT_pool", bufs=1) as vsT_pool, \
                     tc.tile_pool(name="psA", bufs=1, space="PSUM") as psA:
                    # v weights on the second (gpsimd) DMA queue, x on the
                    # sync queue, so wave-1 inputs stream in parallel.
                    wq = {}
                    for nm in ("vw", "qw", "kw"):
                        wt = wqkv_pool.tile([128, CT, C], f32r, name=f"w_{nm}")
                        src = w_in[nm].rearrange("(kt p) o -> p kt o", p=128)
                        for kt in range(CT):
                            nc.scalar.dma_start(wt[:, kt, :], src[:, kt, :])
                        wq[nm] = wt
                    # packed constants: one DMA, before x (biases gate LIF)
                    consts = pers.tile([128, T, 44], f32, name="consts")
                    nc.sync.dma_start(consts[:], consts_e[:, :, :])
                    off = 0
                    for nm, cnt in [("qb", CT), ("kb", CT), ("vb", CT), ("pb", CT),
                                    ("b1", HT), ("b2", CT)]:
                        bias_sb[nm] = (consts, off)
                        off += cnt
                    for nm in ("pA", "fA"):
                        sc_sb[nm] = (consts, off)
                        off += CT

                    def bap(nm, t, ot):
                        tl, o = bias_sb[nm]
                        return tl[:, t - 1, o + ot:o + ot + 1]

                    def sap(nm, t, ot):
                        tl, o = sc_sb[nm]
                        return tl[:, t - 1, o + ot:o + ot + 1]
                    bias_sb["ap"] = bap
                    sc_sb["ap"] = sap
                    # ident / idc: small, on the sync queue before x
                    nc.sync.dma_start(ident16[:], ident16_e[:, :])
                    for _i in range(3):
                        nc.sync.dma_start(idc[_i][:], idc_e[_i])
                    # x: pre-rounded on host, lands straight in f32r tiles
                    xr = {}
                    for t in range(T):
                        for ct in range(CT):
                            xrt = xr_pool.tile([128, N], f32r, name="xr", tag="xr", bufs=16)
                            nc.sync.dma_start(xrt[:], xbr[t, ct])
                            xr[t, ct] = xrt

                    # ---- stage QKV: t-major waves of 12 chains, carry folded
                    # into PSUM; v transpose after each wave.
                    vsT = {}
                    carries = {}
                    for t in range(1, T + 1):
                        for nm, bnm in [("vw", "vb"), ("qw", "qb"), ("kw", "kb")]:
                            for ot in range(CT):
                                ps = psA.tile([128, N], f32, name="psq", tag="psq", bufs=6)
                                carry = carries.get((nm, ot))
                                if carry is not None:
                                    carry_mm(ps, t, carry)
                                for kt in range(CT):
                                    nc.tensor.matmul(ps[:],
                                                     wq[nm][:, kt, ot * 128:(ot + 1) * 128],
                                                     xr[t - 1, kt][:],
                                                     start=(kt == 0 and carry is None),
                                                     stop=(kt == CT - 1))
                                if nm == "vw":
                                    st = vsT_pool.tile([128, N], fp16, name="spk_v",
                                                       tag="spk_v", bufs=8)
                                    vsT[t - 1, ot] = st
                                else:
                                    st = qkvn_pool.tile([128, N], fp8, name=f"spk_{nm}",
                                                        tag=f"spk_{nm}", bufs=16)
                                    if nm == "qw":
                                        qs[t - 1, ot] = st
                                    else:
                                        ks[t - 1, ot] = st
                                carries[nm, ot] = lif_fold(
                                    t, ps, bias_sb["ap"](bnm, t, ot), 1.0,
                                    st[:], f"c_{nm}_{ot}" if t < T else None, xr_pool)
                        # v spikes of this wave -> natural layout (fp16)
                        for nt in range(NT):
                            ps = psA.tile([128, C], f32, name="psv", tag="psx", bufs=2)
                            for ct in range(CT):
                                nc.tensor.matmul(
                                    ps[:, ct * 128:(ct + 1) * 128],
                                    vsT[t - 1, ct][:, nt * 128:(nt + 1) * 128],
                                    ident16[:], start=True, stop=True)
                            vt = qkvn_pool.tile([128, C], fp16, name="vn", tag="vn",
                                                bufs=16)
                            nc.scalar.copy(vt[:], ps[:])
                            vn[t - 1, nt] = vt

                # ---- stage RET (retention): head pairs; out-matmuls run one
                # timestep behind the scores so S*D (on Pool) overlaps the PE.
                with tc.tile_pool(name="dpool", bufs=2) as dpool, \
                     tc.tile_pool(name="spool", bufs=1) as spool, \
                     tc.tile_pool(name="psR", bufs=1, space="PSUM") as psR:
                    # proj + MLP weights loaded here (second DMA queue) so
                    # they land while retention computes.
                    pw8 = {}
                    for j in range(2):
                        wt = pw8_pool.tile([128, CT, C], fp8, name=f"w_pw{j}")
                        nc.scalar.dma_start(
                            wt[:], w_in[f"pw{j}"].rearrange("(kt p) o -> p kt o", p=128))
                        pw8[j] = wt
                    w1r = wmlp_pool.tile([128, CT, HID], f32r, name="w_w1r")
                    nc.scalar.dma_start(
                        w1r[:], w_in["w1"].rearrange("(kt p) o -> p kt o", p=128))
                    w28 = {}
                    for j in range(2):
                        wt = wmlp_pool.tile([128, HT, C], fp8, name=f"w_w2{j}")
                        nc.scalar.dma_start(
                            wt[:], w_in[f"w2{j}"].rearrange("(kt p) o -> p kt o", p=128))
                        w28[j] = wt

                    for hp in range(H // 2):
                        h0, h1 = 2 * hp, 2 * hp + 1
                        dm0 = dpool.tile([128, NT, N], fp16, name="dm0", tag="dm0")
                        nc.sync.dma_start(dm0[:], dmat_e[h0].rearrange("mt p n -> p mt n"))
                        dm1 = dpool.tile([128, NT, N], fp16, name="dm1", tag="dm1")
                        nc.sync.dma_start(dm1[:], dmat_e[h1].rearrange("mt p n -> p mt n"))
                        carry = None
                        stage = []  # (t, stiles) awaiting out-matmul
                        for t in range(T):
                            stiles = []
                            for mt in range(NT):
                                ps0 = psR.tile([128, N], f32, name="ps_s0", tag="ps_s0",
                                               bufs=2)
                                nc.tensor.matmul(ps0[:],
                                                 ks[t, hp][0:64, mt * 128:(mt + 1) * 128],
                                                 qs[t, hp][0:64, :], start=True, stop=True)
                                ps1 = psR.tile([128, N], f32, name="ps_s1", tag="ps_s1",
                                               bufs=2)
                                nc.tensor.matmul(ps1[:],
                                                 ks[t, hp][64:128, mt * 128:(mt + 1) * 128],
                                                 qs[t, hp][64:128, :], start=True, stop=True)
                                s0 = spool.tile([128, N], fp16, name="sd0", tag="sd0",
                                                bufs=6)
                                s1 = spool.tile([128, N], fp16, name="sd1", tag="sd1",
                                                bufs=6)
                                # S*D split ~5:3 between DVE and Pool; the
                                # DVE share goes via an ACT fp16 eviction so
                                # the DVE multiply runs in 2x fp16 mode.
                                def sd(dst, psx, dm_ap, mode):
                                    # GPSIMD cannot read PSUM: Pool work gets
                                    # an ACT fp16 eviction first.
                                    if mode == 2:  # direct DVE from PSUM
                                        nc.vector.tensor_tensor(dst[:], psx[:], dm_ap,
                                                                Alu.mult)
                                        return
                                    sc = spool.tile([128, N], fp16, name="s16",
                                                    tag="s16", bufs=5)
                                    nc.scalar.copy(sc[:], psx[:])
                                    if mode == 1:  # fast fp16 DVE
                                        nc.vector.tensor_tensor(dst[:], sc[:], dm_ap,
                                                                Alu.mult)
                                    else:  # Pool, SBUF-only
                                        nc.gpsimd.tensor_tensor(dst[:], sc[:], dm_ap,
                                                                Alu.mult)
                                m0, m1 = [(2, 2), (2, 1), (1, 1), (0, 0)][mt]
                                sd(s0, ps0, dm0[:, mt, :], m0)
                                sd(s1, ps1, dm1[:, mt, :], m1)
                                stiles.append((s0, s1))
                            stage.append((t, stiles))
                            if len(stage) == 2 or t == T - 1:
                                for tt_, stl in stage if t == T - 1 else stage[:1]:
                                    pso = psR.tile([128, N], f32, name="ps_o", tag="ps_o",
                                                   bufs=3)
                                    if carry is not None:
                                        carry_mm(pso, tt_ + 1, carry)
                                    for mt in range(NT):
                                        s0, s1 = stl[mt]
                                        nc.tensor.matmul(
                                            pso[0:64, :],
                                            vn[tt_, mt][:, h0 * 64:(h0 + 1) * 64], s0[:],
                                            start=(mt == 0 and carry is None), stop=False)
                                        nc.tensor.matmul(
                                            pso[64:128, :],
                                            vn[tt_, mt][:, h1 * 64:(h1 + 1) * 64], s1[:],
                                            start=(mt == 0 and carry is None),
                                            stop=(mt == NT - 1))
                                    carry = lif_fold(
                                        tt_ + 1, pso, None, 0.5,
                                        os8[tt_, hp // 2][:, hp % 2, :],
                                        "c_o" if tt_ < T - 1 else None, spool)
                                stage = stage[1:] if t < T - 1 else []

            # ---- merged PROJ + MLP, t-outer; fc2 runs one timestep behind.
            with tc.tile_pool(name="mwork", bufs=1) as mwork, \
                 tc.tile_pool(name="xtin_pool", bufs=1) as xtin_pool, \
                 tc.tile_pool(name="psM", bufs=1, space="PSUM") as psM:
                cp = {}
                c1 = {}
                c2 = {}
                x2 = {}
                h8 = {}
                fc2_pend = []

                def emit_fc2(t):
                    # fc2 fp8-DR 2-term for timestep t (1-based)
                    for ot in range(CT):
                        psa = psM.tile([128, N], f32, name="psf2A", tag="psf2A", bufs=2)
                        for j in range(HT // 2):
                            for trm in (0, 1):
                                nc.tensor.matmul(
                                    psa[:],
                                    w28[trm][:, 2 * j:2 * j + 2, ot * 128:(ot + 1) * 128],
                                    h8[t, j][:, :, :],
                                    start=(j == 0 and trm == 0),
                                    stop=(j == HT // 2 - 1 and trm == 1),
                                    perf_mode=DR)
                        stm = mwork.tile([128, N], fp16, name="spk_m", tag="spk_m", bufs=2)
                        c2[ot] = lif_pool(t, psa, bias_sb["ap"]("b2", t, ot),
                                          sc_sb["ap"]("fA", t, ot),
                                          c2.get(ot), stm[:],
                                          f"c2_{ot}" if t < T else None, mwork)
                        ot_t = mwork.tile([128, N], f32, name="outT", tag="outT", bufs=2)
                        nc.gpsimd.tensor_tensor(ot_t[:], x2[t, ot][:], stm[:], Alu.add)
                        nc.sync.dma_start(out_e[t - 1, ot], ot_t[:])

                for t in range(1, T + 1):
                    # proj fp8-DR 2-term + LIF -> attn spikes; x2 = x + attn
                    for ot in range(CT):
                        psa = psM.tile([128, N], f32, name="pspA", tag="pspA", bufs=2)
                        for j in range(CT // 2):
                            for trm in (0, 1):
                                nc.tensor.matmul(
                                    psa[:],
                                    pw8[trm][:, 2 * j:2 * j + 2, ot * 128:(ot + 1) * 128],
                                    os8[t - 1, j][:, :, :],
                                    start=(j == 0 and trm == 0),
                                    stop=(j == CT // 2 - 1 and trm == 1),
                                    perf_mode=DR)
                        stp = mwork.tile([128, N], fp16, name="spk_p", tag="spk_p", bufs=4)
                        cp[ot] = lif_pool(t, psa, bias_sb["ap"]("pb", t, ot),
                                          sc_sb["ap"]("pA", t, ot),
                                          cp.get(ot), stp[:],
                                          f"cp_{ot}" if t < T else None, mwork)
                        xt = xtin_pool.tile([128, N], f32, name="xtin", tag="xtin", bufs=2)
                        nc.sync.dma_start(xt[:], xb[t - 1, ot])
                        x2t = mwork.tile([128, N], f32, name="x2t", tag="x2t", bufs=8)
                        nc.gpsimd.tensor_tensor(x2t[:], xt[:], stp[:], Alu.add)
                        x2[t, ot] = x2t
                        x2r = mwork.tile([128, N], f32r, name="x2r", tag="x2r", bufs=5)
                        nc.scalar.copy(x2r[:], x2t[:])
                        x2[t, ot, "r"] = x2r

                    # fc2 of previous timestep fills the PE while fc1 LIF drains
                    if fc2_pend:
                        emit_fc2(fc2_pend.pop())

                    # fc1 f32r single pass + LIF -> h spikes (fp8 pair tiles)
                    for ot in range(HT):
                        ps = psM.tile([128, N], f32, name="psf1", tag="psf1", bufs=4)
                        carry = c1.get(ot)
                        if carry is not None:
                            carry_mm(ps, t, carry)
                        for kt in range(CT):
                            nc.tensor.matmul(ps[:], w1r[:, kt, ot * 128:(ot + 1) * 128],
                                             x2[t, kt, "r"][:],
                                             start=(kt == 0 and carry is None),
                                             stop=(kt == CT - 1))
                        if ot % 2 == 0:
                            h8[t, ot // 2] = mwork.tile([128, 2, N], fp8, name="spk_h",
                                                        tag="spk_h", bufs=16)
                        c1[ot] = lif_fold(t, ps, bias_sb["ap"]("b1", t, ot), 1.0,
                                          h8[t, ot // 2][:, ot % 2, :],
                                          f"c1_{ot}" if t < T else None, mwork)
                    fc2_pend.append(t)

                emit_fc2(fc2_pend.pop())

    nc.finalize()
    return nc


def _round12(x):
    x = np.ascontiguousarray(x, np.float32)
    m = x.view(np.uint32)
    lsb = (m >> 12) & 1
    m = (m + 0x7FF + lsb) & np.uint32(0xFFFFF000)
    return m.view(np.float32)


def _host_prep(inputs):
    def fold(w, b, bn):
        g, bb, m, v = [bn[i].astype(np.float64) for i in range(4)]
        A = g / np.sqrt(v + EPS)
        W = w.astype(np.float64) * A[:, None]
        bias = (b.astype(np.float64) - m) * A + bb
        return W, bias

    def bias_layout(bias):
        co = bias.shape[0]
        arr = np.stack([(bias * (2.0 ** t)).reshape(co // 128, 128).T
                        for t in range(T)], axis=1)
        return np.ascontiguousarray(arr.astype(np.float32))

    def split8(W):
        """Per-row 2-term e4m3 split, single pow2 scale per row."""
        e4 = ml_dtypes.float8_e4m3
        rmaxA = np.abs(W).max(axis=1)
        sA = 2.0 ** np.floor(np.log2(208.0 / np.maximum(rmaxA, 1e-30)))
        Ws = W * sA[:, None]
        q0 = Ws.astype(e4)
        e0 = Ws - q0.astype(np.float64)
        q1 = e0.astype(e4)
        for q in (q0, q1):
            assert np.isfinite(q.astype(np.float64)).all()
        return (q0, q1), sA

    def scale_layout(s):
        co = s.shape[0]
        arr = np.stack([(s * (2.0 ** t)).reshape(co // 128, 128).T
                        for t in range(T)], axis=1)
        return np.ascontiguousarray(arr.astype(np.float32))

    feed = {}
    packed = {}
    for wkey, bkey, bnkey, bout in [("qw", "qb", "qbn", "qb"), ("kw", "kb", "kbn", "kb"),
                                    ("vw", "vb", "vbn", "vb"), ("w1", "b1", "bn1", "b1")]:
        W, bias = fold(inputs[wkey], inputs[bkey], inputs[bnkey])
        feed[wkey + "r"] = _round12(np.ascontiguousarray(W.T, np.float32))
        packed[bout] = bias_layout(bias)
    for wkey, bkey, bnkey, bout, sa in [("pw", "pb", "pbn", "pb", "pA"),
                                        ("w2", "b2", "bn2", "b2", "fA")]:
        W, bias = fold(inputs[wkey], inputs[bkey], inputs[bnkey])
        (q0, q1), sA = split8(W)
        for j, q in enumerate((q0, q1)):
            feed[f"{wkey}{j}"] = np.ascontiguousarray(q.T).view(np.uint8)
        packed[bout] = bias_layout(bias)
        packed["sc" + sa] = scale_layout(1.0 / sA)
    feed["consts"] = np.ascontiguousarray(np.concatenate(
        [packed[k] for k in ("qb", "kb", "vb", "pb", "b1", "b2", "scpA", "scfA")],
        axis=2))

    gamma = 1.0 - 2.0 ** (-5.0 - np.arange(H, dtype=np.float64))
    idx = np.arange(N, dtype=np.float64)
    dist = np.abs(idx[:, None] - idx[None, :])
    scale = (C // H) ** -0.5
    dm = np.empty((H, NT, 128, N), np.float16)
    for h in range(H):
        dm[h] = ((gamma[h] ** dist) * scale).reshape(NT, 128, N).astype(np.float16)
    feed["dmat"] = dm
    feed["ident16"] = np.eye(128, dtype=np.float16)
    idc = np.zeros((3, 128, 128), np.float32)
    for i, t in enumerate(range(2, T + 1)):
        idc[i] = np.eye(128, dtype=np.float32) * (2.0 ** -(t - 1))
    feed["idc"] = idc
    return feed


def kernel(**inputs):
    if "nc" not in _CACHE:
        _CACHE["nc"] = _build()
    nc = _CACHE["nc"]
    feed = _host_prep(inputs)
    x = inputs["x"]
    in_maps = []
    for b in range(B):
        m = dict(feed)
        xt = np.ascontiguousarray(x[:, b].transpose(0, 2, 1).reshape(T, CT, 128, N))
        m["xb"] = xt
        m["xbr"] = _round12(xt)
        in_maps.append(m)
    res = None
    last_err = None
    for _attempt in range(3):
        try:
            res = run_bass_kernel_spmd(nc, in_maps, list(range(B)))
            break
        except Exception as e:  # transient NRT device wedges recover on retry
            last_err = e
    if res is None:
        raise last_err
    out = np.empty((T, B, N, C), np.float32)
    for b in range(B):
        oT = res.results[b]["out"].reshape(T, C, N)
        out[:, b] = oT.transpose(0, 2, 1)
    return out


# revision 52
# speedup vs baseline: 2.3950x; 1.0371x over previous
"""Spiking transformer block (SpikingRetention + spiking MLP) on 8 Trainium2 cores.

Data-parallel over B=8 (one batch element per NeuronCore), weights replicated.

Activations are feature-major ([C partitions, N free]); folded BatchNorm is a
per-partition affine on the ACT engine. LIF membranes are 2^t-scaled
(A_t = 2^t u_t) so each step is compare / masked-reset with exact power-of-2
constants.

Matmul plan (TRN2 PE cost: cycles = out_free x cycles_per_row):
  - q/k/v, fc1 (dense x dense): single-pass float32r (both operands rounded
    to 12-bit mantissa; 1 cycle/row at free>=256). x is pre-rounded on host.
  - proj, fc2 (binary x dense): fp8e4 DoubleRow (0.5 cycles/row, 2 k-tiles
    per instruction), weights as 2 e4m3 terms with per-row pow2 scales.
  - scores: fp8e4 x fp8e4 spikes, exact integer accumulation.
  - (S*D)@v: fp16 throughout (S exact ints in fp16, D fp16, v binary fp16).

LIF: the carried membrane is folded into the PSUM accumulation with a tiny
diagonal matmul (carry * 2^-(t-1) added by the PE), so each step is just
ACT affine (PSUM->SBUF) + DVE spike (is_ge) + DVE masked-reset carry.
The fp8 layers use per-row scales, so their carry is added on Pool instead.
S*D products are split across DVE (direct from PSUM), ACT-evict + fp16 DVE
(2x mode), and ACT-evict + Pool, to balance all four engines in retention.
"""

from contextlib import ExitStack

import numpy as np
import ml_dtypes

import concourse.bacc as bacc
import concourse.tile as tile
from concourse import mybir
from concourse.bass_utils import run_bass_kernel_spmd

f32 = mybir.dt.float32
f32r = mybir.dt.float32r
fp16 = mybir.dt.float16
fp8 = mybir.dt.float8e4
Alu = mybir.AluOpType
Act = mybir.ActivationFunctionType
DR = mybir.MatmulPerfMode.DoubleRow

T, B, N, C = 4, 8, 512, 512
HID = 2048
H = 8
EPS = 1e-5
NT = N // 128
CT = C // 128
HT = HID // 128

_CACHE = {}


def _build():
    nc = bacc.Bacc("TRN2", target_bir_lowering=False, debug=False)

    xbr = nc.declare_dram_parameter("xbr", [T, CT, 128, N], f32r, isOutput=False)
    xb = nc.declare_dram_parameter("xb", [T, CT, 128, N], f32, isOutput=False)
    ident16_e = nc.declare_dram_parameter("ident16", [128, 128], fp16, isOutput=False)
    # diag(2^-(t-1)) for folding the LIF carry into PSUM, t = 2..4
    idc_e = nc.declare_dram_parameter("idc", [3, 128, 128], f32r, isOutput=False)
    w_in = {}
    for nm, ci, co in [("qw", C, C), ("kw", C, C), ("vw", C, C), ("w1", C, HID)]:
        w_in[nm] = nc.declare_dram_parameter(nm + "r", [ci, co], f32r, isOutput=False)
    for nm, ci, co in [("pw", C, C), ("w2", HID, C)]:
        for j in range(2):
            w_in[f"{nm}{j}"] = nc.declare_dram_parameter(f"{nm}{j}", [ci, co], fp8,
                                                         isOutput=False)
    consts_e = nc.declare_dram_parameter("consts", [128, T, 44], f32, isOutput=False)
    dmat_e = nc.declare_dram_parameter("dmat", [H, NT, 128, N], fp16, isOutput=False)
    out_e = nc.declare_dram_parameter("out", [T, CT, 128, N], f32, isOutput=True)

    with tile.TileContext(nc) as tc, ExitStack() as ctx:
        pers = ctx.enter_context(tc.tile_pool(name="pers", bufs=1))
        work = ctx.enter_context(tc.tile_pool(name="work", bufs=1))

        ident16 = pers.tile([128, 128], fp16, name="ident16")
        idc = [pers.tile([128, 128], f32r, name=f"idc{t}") for t in (2, 3, 4)]
        bias_sb = {}
        sc_sb = {}

        def carry_mm(ps, t, carry):
            """Fold carry*2^-(t-1) into a PSUM accumulation group (opens it)."""
            nc.tensor.matmul(ps[:], idc[t - 2][:], carry[:], start=True, stop=False)

        def lif_fold(t, ps, bias_ap, theta, spike_ap, carry_tag, cpool):
            """Carry already folded in PSUM. A = 2^(t-1)*ps + bias (ACT);
            spike (DVE); carry' = A*(A<thr) in f32r (DVE)."""
            sc = float(2.0 ** (t - 1))
            A = work.tile([128, N], f32, name="lifA", tag="lifA", bufs=6)
            if bias_ap is None:
                nc.scalar.activation(A[:], ps[:], Act.Copy, bias=0.0, scale=sc)
            else:
                nc.scalar.activation(A[:], ps[:], Act.Identity, bias=bias_ap, scale=sc)
            thr = float(theta * (2.0 ** t))
            nc.vector.tensor_scalar(spike_ap, A[:], thr, None, Alu.is_ge)
            if carry_tag is not None:
                Cn = cpool.tile([128, N], f32r, name="lifC", tag=carry_tag, bufs=1)
                nc.vector.scalar_tensor_tensor(Cn[:], A[:], thr, A[:], Alu.is_lt, Alu.mult)
                return Cn
            return None

        def lif_pool(t, ps, bias_ap, scale_ap, carry_in, spike_ap, carry_tag, cpool):
            """fp8-layer path (per-row scale): zb = scale*ps + bias (ACT);
            A = zb + carry (Pool); spike / carry (DVE)."""
            zb = work.tile([128, N], f32, name="lifzb", tag="lifzb", bufs=4)
            nc.scalar.activation(zb[:], ps[:], Act.Identity, bias=bias_ap, scale=scale_ap)
            if carry_in is None:
                A = zb
            else:
                A = work.tile([128, N], f32, name="lifA", tag="lifA", bufs=6)
                nc.gpsimd.tensor_tensor(A[:], carry_in[:], zb[:], Alu.add)
            thr = float(2.0 ** t)
            nc.vector.tensor_scalar(spike_ap, A[:], thr, None, Alu.is_ge)
            if carry_tag is not None:
                Cn = cpool.tile([128, N], f32, name="lifCp", tag=carry_tag, bufs=1)
                nc.vector.scalar_tensor_tensor(Cn[:], A[:], thr, A[:], Alu.is_lt, Alu.mult)
                return Cn
            return None

        with tc.tile_pool(name="pw8_pool", bufs=1) as pw8_pool, \
             tc.tile_pool(name="spk_o_pool", bufs=1) as spk_o_pool, \
             tc.tile_pool(name="wmlp_pool", bufs=1) as wmlp_pool:
            # o spikes: fp8 pair tiles [128, 2, N]; j-th tile holds head-pairs 2j, 2j+1
            os8 = {}
            for t in range(T):
                for j in range(2):
                    os8[t, j] = spk_o_pool.tile([128, 2, N], fp8, name=f"os8_{t}_{j}")

            with tc.tile_pool(name="qkvn_pool", bufs=1) as qkvn_pool:
                qs = {}
                ks = {}
                vn = {}
                with tc.tile_pool(name="wqkv_pool", bufs=1) as wqkv_pool, \
                     tc.tile_pool(name="xr_pool", bufs=1) as xr_pool, \
                     tc.tile_pool(name="vsT_pool", bufs=1) as vsT_pool, \
                     tc.tile_pool(name="psA", bufs=1, space="PSUM") as psA:
                    # v weights on the second (gpsimd) DMA queue, x on the
                    # sync queue, so wave-1 inputs stream in parallel.
                    wq = {}
                    for nm in ("vw", "qw", "kw"):
                        wt = wqkv_pool.tile([128, CT, C], f32r, name=f"w_{nm}")
                        src = w_in[nm].rearrange("(kt p) o -> p kt o", p=128)
                        for kt in range(CT):
                            nc.scalar.dma_start(wt[:, kt, :], src[:, kt, :])
                        wq[nm] = wt
                    # packed constants: one DMA, before x (biases gate LIF)
                    consts = pers.tile([128, T, 44], f32, name="consts")
                    nc.sync.dma_start(consts[:], consts_e[:, :, :])
                    off = 0
                    for nm, cnt in [("qb", CT), ("kb", CT), ("vb", CT), ("pb", CT),
                                    ("b1", HT), ("b2", CT)]:
                        bias_sb[nm] = (consts, off)
                        off += cnt
                    for nm in ("pA", "fA"):
                        sc_sb[nm] = (consts, off)
                        off += CT

                    def bap(nm, t, ot):
                        tl, o = bias_sb[nm]
                        return tl[:, t - 1, o + ot:o + ot + 1]

                    def sap(nm, t, ot):
                        tl, o = sc_sb[nm]
                        return tl[:, t - 1, o + ot:o + ot + 1]
                    bias_sb["ap"] = bap
                    sc_sb["ap"] = sap
                    # x: pre-rounded on host, lands straight in f32r tiles;
                    # ident/idc go on the queue after the first wave's inputs
                    xr = {}
                    for t in range(T):
                        for ct in range(CT):
                            xrt = xr_pool.tile([128, N], f32r, name="xr", tag="xr", bufs=16)
                            nc.sync.dma_start(xrt[:], xbr[t, ct])
                            xr[t, ct] = xrt
                        if t == 0:
                            nc.sync.dma_start(ident16[:], ident16_e[:, :])
                            for _i in range(3):
                                nc.sync.dma_start(idc[_i][:], idc_e[_i])

                    # ---- stage QKV: t-major waves of 12 chains, carry folded
                    # into PSUM; v transpose after each wave.
                    vsT = {}
                    carries = {}
                    for t in range(1, T + 1):
                        for nm, bnm in [("vw", "vb"), ("qw", "qb"), ("kw", "kb")]:
                            for ot in range(CT):
                                ps = psA.tile([128, N], f32, name="psq", tag="psq", bufs=6)
                                carry = carries.get((nm, ot))
                                if carry is not None:
                                    carry_mm(ps, t, carry)
                                for kt in range(CT):
                                    nc.tensor.matmul(ps[:],
                                                     wq[nm][:, kt, ot * 128:(ot + 1) * 128],
                                                     xr[t - 1, kt][:],
                                                     start=(kt == 0 and carry is None),
                                                     stop=(kt == CT - 1))
                                if nm == "vw":
                                    st = vsT_pool.tile([128, N], fp16, name="spk_v",
                                                       tag="spk_v", bufs=8)
                                    vsT[t - 1, ot] = st
                                else:
                                    st = qkvn_pool.tile([128, N], fp8, name=f"spk_{nm}",
                                                        tag=f"spk_{nm}", bufs=16)
                                    if nm == "qw":
                                        qs[t - 1, ot] = st
                                    else:
                                        ks[t - 1, ot] = st
                                carries[nm, ot] = lif_fold(
                                    t, ps, bias_sb["ap"](bnm, t, ot), 1.0,
                                    st[:], f"c_{nm}_{ot}" if t < T else None, xr_pool)
                        # v spikes of this wave -> natural layout (fp16)
                        for nt in range(NT):
                            ps = psA.tile([128, C], f32, name="psv", tag="psx", bufs=2)
                            for ct in range(CT):
                                nc.tensor.matmul(
                                    ps[:, ct * 128:(ct + 1) * 128],
                                    vsT[t - 1, ct][:, nt * 128:(nt + 1) * 128],
                                    ident16[:], start=True, stop=True)
                            vt = qkvn_pool.tile([128, C], fp16, name="vn", tag="vn",
                                                bufs=16)
                            nc.scalar.copy(vt[:], ps[:])
                            vn[t - 1, nt] = vt

                # ---- stage RET (retention): head pairs; out-matmuls run one
                # timestep behind the scores so S*D (on Pool) overlaps the PE.
                with tc.tile_pool(name="dpool", bufs=2) as dpool, \
                     tc.tile_pool(name="spool", bufs=1) as spool, \
                     tc.tile_pool(name="psR", bufs=1, space="PSUM") as psR:
                    # proj + MLP weights loaded here (second DMA queue) so
                    # they land while retention computes.
                    pw8 = {}
                    for j in range(2):
                        wt = pw8_pool.tile([128, CT, C], fp8, name=f"w_pw{j}")
                        nc.scalar.dma_start(
                            wt[:], w_in[f"pw{j}"].rearrange("(kt p) o -> p kt o", p=128))
                        pw8[j] = wt
                    w1r = wmlp_pool.tile([128, CT, HID], f32r, name="w_w1r")
                    nc.scalar.dma_start(
                        w1r[:], w_in["w1"].rearrange("(kt p) o -> p kt o", p=128))
                    w28 = {}
                    for j in range(2):
                        wt = wmlp_pool.tile([128, HT, C], fp8, name=f"w_w2{j}")
                        nc.scalar.dma_start(
                            wt[:], w_in[f"w2{j}"].rearrange("(kt p) o -> p kt o", p=128))
                        w28[j] = wt

                    for hp in range(H // 2):
                        h0, h1 = 2 * hp, 2 * hp + 1
                        dm0 = dpool.tile([128, NT, N], fp16, name="dm0", tag="dm0")
                        nc.sync.dma_start(dm0[:], dmat_e[h0].rearrange("mt p n -> p mt n"))
                        dm1 = dpool.tile([128, NT, N], fp16, name="dm1", tag="dm1")
                        nc.sync.dma_start(dm1[:], dmat_e[h1].rearrange("mt p n -> p mt n"))
                        carry = None
                        stage = []  # (t, stiles) awaiting out-matmul
                        for t in range(T):
                            stiles = []
                            for mt in range(NT):
                                ps0 = psR.tile([128, N], f32, name="ps_s0", tag="ps_s0",
                                               bufs=3)
                                nc.tensor.matmul(ps0[:],
                                                 ks[t, hp][0:64, mt * 128:(mt + 1) * 128],
                                                 qs[t, hp][0:64, :], start=True, stop=True)
                                ps1 = psR.tile([128, N], f32, name="ps_s1", tag="ps_s1",
                                               bufs=3)
                                nc.tensor.matmul(ps1[:],
                                                 ks[t, hp][64:128, mt * 128:(mt + 1) * 128],
                                                 qs[t, hp][64:128, :], start=True, stop=True)
                                s0 = spool.tile([128, N], fp16, name="sd0", tag="sd0",
                                                bufs=8)
                                s1 = spool.tile([128, N], fp16, name="sd1", tag="sd1",
                                                bufs=8)
                                # S*D split ~5:3 between DVE and Pool; the
                                # DVE share goes via an ACT fp16 eviction so
                                # the DVE multiply runs in 2x fp16 mode.
                                def sd(dst, psx, dm_ap, mode):
                                    # GPSIMD cannot read PSUM: Pool work gets
                                    # an ACT fp16 eviction first.
                                    if mode == 2:  # direct DVE from PSUM
                                        nc.vector.tensor_tensor(dst[:], psx[:], dm_ap,
                                                                Alu.mult)
                                        return
                                    sc = spool.tile([128, N], fp16, name="s16",
                                                    tag="s16", bufs=5)
                                    nc.scalar.copy(sc[:], psx[:])
                                    if mode == 1:  # fast fp16 DVE
                                        nc.vector.tensor_tensor(dst[:], sc[:], dm_ap,
                                                                Alu.mult)
                                    else:  # Pool, SBUF-only
                                        nc.gpsimd.tensor_tensor(dst[:], sc[:], dm_ap,
                                                                Alu.mult)
                                m0, m1 = [(2, 2), (2, 1), (1, 1), (0, 0)][mt]
                                sd(s0, ps0, dm0[:, mt, :], m0)
                                sd(s1, ps1, dm1[:, mt, :], m1)
                                stiles.append((s0, s1))
                            stage.append((t, stiles))
                            if len(stage) == 2 or t == T - 1:
                                for tt_, stl in stage if t == T - 1 else stage[:1]:
                                    pso = psR.tile([128, N], f32, name="ps_o", tag="ps_o",
                                                   bufs=2)
                                    if carry is not None:
                                        carry_mm(pso, tt_ + 1, carry)
                                    for mt in range(NT):
                                        s0, s1 = stl[mt]
                                        nc.tensor.matmul(
                                            pso[0:64, :],
                                            vn[tt_, mt][:, h0 * 64:(h0 + 1) * 64], s0[:],
                                            start=(mt == 0 and carry is None), stop=False)
                                        nc.tensor.matmul(
                                            pso[64:128, :],
                                            vn[tt_, mt][:, h1 * 64:(h1 + 1) * 64], s1[:],
                                            start=(mt == 0 and carry is None),
                                            stop=(mt == NT - 1))
                                    carry = lif_fold(
                                        tt_ + 1, pso, None, 0.5,
                                        os8[tt_, hp // 2][:, hp % 2, :],
                                        "c_o" if tt_ < T - 1 else None, spool)
                                stage = stage[1:] if t < T - 1 else []

            # ---- merged PROJ + MLP, t-outer; fc2 runs one timestep behind.
            with tc.tile_pool(name="mwork", bufs=1) as mwork, \
                 tc.tile_pool(name="xtin_pool", bufs=1) as xtin_pool, \
                 tc.tile_pool(name="psM", bufs=1, space="PSUM") as psM:
                cp = {}
                c1 = {}
                c2 = {}
                x2 = {}
                h8 = {}
                fc2_pend = []

                def emit_fc2(t):
                    # fc2 fp8-DR 2-term for timestep t (1-based)
                    for ot in range(CT):
                        psa = psM.tile([128, N], f32, name="psf2A", tag="psf2A", bufs=2)
                        for j in range(HT // 2):
                            for trm in (0, 1):
                                nc.tensor.matmul(
                                    psa[:],
                                    w28[trm][:, 2 * j:2 * j + 2, ot * 128:(ot + 1) * 128],
                                    h8[t, j][:, :, :],
                                    start=(j == 0 and trm == 0),
                                    stop=(j == HT // 2 - 1 and trm == 1),
                                    perf_mode=DR)
                        stm = mwork.tile([128, N], fp16, name="spk_m", tag="spk_m", bufs=2)
                        c2[ot] = lif_pool(t, psa, bias_sb["ap"]("b2", t, ot),
                                          sc_sb["ap"]("fA", t, ot),
                                          c2.get(ot), stm[:],
                                          f"c2_{ot}" if t < T else None, mwork)
                        ot_t = mwork.tile([128, N], f32, name="outT", tag="outT", bufs=2)
                        nc.gpsimd.tensor_tensor(ot_t[:], x2[t, ot][:], stm[:], Alu.add)
                        nc.sync.dma_start(out_e[t - 1, ot], ot_t[:])

                for t in range(1, T + 1):
                    # proj fp8-DR 2-term + LIF -> attn spikes; x2 = x + attn
                    for ot in range(CT):
                        psa = psM.tile([128, N], f32, name="pspA", tag="pspA", bufs=2)
                        for j in range(CT // 2):
                            for trm in (0, 1):
                                nc.tensor.matmul(
                                    psa[:],
                                    pw8[trm][:, 2 * j:2 * j + 2, ot * 128:(ot + 1) * 128],
                                    os8[t - 1, j][:, :, :],
                                    start=(j == 0 and trm == 0),
                                    stop=(j == CT // 2 - 1 and trm == 1),
                                    perf_mode=DR)
                        stp = mwork.tile([128, N], fp16, name="spk_p", tag="spk_p", bufs=4)
                        cp[ot] = lif_pool(t, psa, bias_sb["ap"]("pb", t, ot),
                                          sc_sb["ap"]("pA", t, ot),
                                          cp.get(ot), stp[:],
                                          f"cp_{ot}" if t < T else None, mwork)
                        xt = xtin_pool.tile([128, N], f32, name="xtin", tag="xtin", bufs=2)
                        nc.sync.dma_start(xt[:], xb[t - 1, ot])
                        x2t = mwork.tile([128, N], f32, name="x2t", tag="x2t", bufs=8)
                        nc.gpsimd.tensor_tensor(x2t[:], xt[:], stp[:], Alu.add)
                        x2[t, ot] = x2t
                        x2r = mwork.tile([128, N], f32r, name="x2r", tag="x2r", bufs=5)
                        nc.scalar.copy(x2r[:], x2t[:])
                        x2[t, ot, "r"] = x2r

                    # fc2 of previous timestep fills the PE while fc1 LIF drains
                    if fc2_pend:
                        emit_fc2(fc2_pend.pop())

                    # fc1 f32r single pass + LIF -> h spikes (fp8 pair tiles)
                    for ot in range(HT):
                        ps = psM.tile([128, N], f32, name="psf1", tag="psf1", bufs=4)
                        carry = c1.get(ot)
                        if carry is not None:
                            carry_mm(ps, t, carry)
                        for kt in range(CT):
                            nc.tensor.matmul(ps[:], w1r[:, kt, ot * 128:(ot + 1) * 128],
                                             x2[t, kt, "r"][:],
                                             start=(kt == 0 and carry is None),
                                             stop=(kt == CT - 1))
                        if ot % 2 == 0:
                            h8[t, ot // 2] = mwork.tile([128, 2, N], fp8, name="spk_h",
                                                        tag="spk_h", bufs=16)
                        c1[ot] = lif_fold(t, ps, bias_sb["ap"]("b1", t, ot), 1.0,
                                          h8[t, ot // 2][:, ot % 2, :],
                                          f"c1_{ot}" if t < T else None, mwork)
                    fc2_pend.append(t)

                emit_fc2(fc2_pend.pop())

    nc.finalize()
    return nc


def _round12(x):
    x = np.ascontiguousarray(x, np.float32)
    m = x.view(np.uint32)
    lsb = (m >> 12) & 1
    m = (m + 0x7FF + lsb) & np.uint32(0xFFFFF000)
    return m.view(np.float32)


def _host_prep(inputs):
    def fold(w, b, bn):
        g, bb, m, v = [bn[i].astype(np.float64) for i in range(4)]
        A = g / np.sqrt(v + EPS)
        W = w.astype(np.float64) * A[:, None]
        bias = (b.astype(np.float64) - m) * A + bb
        return W, bias

    def bias_layout(bias):
        co = bias.shape[0]
        arr = np.stack([(bias * (2.0 ** t)).reshape(co // 128, 128).T
                        for t in range(T)], axis=1)
        return np.ascontiguousarray(arr.astype(np.float32))

    def split8(W):
        """Per-row 2-term e4m3 split, single pow2 scale per row."""
        e4 = ml_dtypes.float8_e4m3
        rmaxA = np.abs(W).max(axis=1)
        sA = 2.0 ** np.floor(np.log2(208.0 / np.maximum(rmaxA, 1e-30)))
        Ws = W * sA[:, None]
        q0 = Ws.astype(e4)
        e0 = Ws - q0.astype(np.float64)
        q1 = e0.astype(e4)
        for q in (q0, q1):
            assert np.isfinite(q.astype(np.float64)).all()
        return (q0, q1), sA

    def scale_layout(s):
        co = s.shape[0]
        arr = np.stack([(s * (2.0 ** t)).reshape(co // 128, 128).T
                        for t in range(T)], axis=1)
        return np.ascontiguousarray(arr.astype(np.float32))

    feed = {}
    packed = {}
    for wkey, bkey, bnkey, bout in [("qw", "qb", "qbn", "qb"), ("kw", "kb", "kbn", "kb"),
                                    ("vw", "vb", "vbn", "vb"), ("w1", "b1", "bn1", "b1")]:
        W, bias = fold(inputs[wkey], inputs[bkey], inputs[bnkey])
        feed[wkey + "r"] = _round12(np.ascontiguousarray(W.T, np.float32))
        packed[bout] = bias_layout(bias)
    for wkey, bkey, bnkey, bout, sa in [("pw", "pb", "pbn", "pb", "pA"),
                                        ("w2", "b2", "bn2", "b2", "fA")]:
        W, bias = fold(inputs[wkey], inputs[bkey], inputs[bnkey])
        (q0, q1), sA = split8(W)
        for j, q in enumerate((q0, q1)):
            feed[f"{wkey}{j}"] = np.ascontiguousarray(q.T).view(np.uint8)
        packed[bout] = bias_layout(bias)
        packed["sc" + sa] = scale_layout(1.0 / sA)
    feed["consts"] = np.ascontiguousarray(np.concatenate(
        [packed[k] for k in ("qb", "kb", "vb", "pb", "b1", "b2", "scpA", "scfA")],
        axis=2))

    gamma = 1.0 - 2.0 ** (-5.0 - np.arange(H, dtype=np.float64))
    idx = np.arange(N, dtype=np.float64)
    dist = np.abs(idx[:, None] - idx[None, :])
    scale = (C // H) ** -0.5
    dm = np.empty((H, NT, 128, N), np.float16)
    for h in range(H):
        dm[h] = ((gamma[h] ** dist) * scale).reshape(NT, 128, N).astype(np.float16)
    feed["dmat"] = dm
    feed["ident16"] = np.eye(128, dtype=np.float16)
    idc = np.zeros((3, 128, 128), np.float32)
    for i, t in enumerate(range(2, T + 1)):
        idc[i] = np.eye(128, dtype=np.float32) * (2.0 ** -(t - 1))
    feed["idc"] = idc
    return feed


def kernel(**inputs):
    if "nc" not in _CACHE:
        _CACHE["nc"] = _build()
    nc = _CACHE["nc"]
    feed = _host_prep(inputs)
    x = inputs["x"]
    in_maps = []
    for b in range(B):
        m = dict(feed)
        xt = np.ascontiguousarray(x[:, b].transpose(0, 2, 1).reshape(T, CT, 128, N))
        m["xb"] = xt
        m["xbr"] = _round12(xt)
        in_maps.append(m)
    res = None
    last_err = None
    for _attempt in range(3):
        try:
            res = run_bass_kernel_spmd(nc, in_maps, list(range(B)))
            break
        except Exception as e:  # transient NRT device wedges recover on retry
            last_err = e
    if res is None:
        raise last_err
    out = np.empty((T, B, N, C), np.float32)
    for b in range(B):
        oT = res.results[b]["out"].reshape(T, C, N)
        out[:, b] = oT.transpose(0, 2, 1)
    return out


# revision 57
# speedup vs baseline: 2.5311x; 1.0568x over previous
"""Spiking transformer block (SpikingRetention + spiking MLP) on 8 Trainium2 cores.

Data-parallel over B=8 (one batch element per NeuronCore), weights replicated.

Activations are feature-major ([C partitions, N free]); folded BatchNorm is a
per-partition affine on the ACT engine. LIF membranes are 2^t-scaled
(A_t = 2^t u_t) so each step is compare / masked-reset with exact power-of-2
constants.

Matmul plan (TRN2 PE cost: cycles = out_free x cycles_per_row):
  - q/k/v, fc1 (dense x dense): single-pass float32r (both operands rounded
    to 12-bit mantissa; 1 cycle/row at free>=256). x is pre-rounded on host.
  - proj, fc2 (binary x dense): fp8e4 DoubleRow (0.5 cycles/row, 2 k-tiles
    per instruction), weights as 2 e4m3 terms with per-row pow2 scales.
  - scores: fp8e4 x fp8e4 spikes, exact integer accumulation.
  - (S*D)@v: fp16 throughout (S exact ints in fp16, D fp16, v binary fp16).

LIF: the carried membrane is folded into the PSUM accumulation with a tiny
diagonal matmul (carry * 2^-(t-1) added by the PE), so each step is just
ACT affine (PSUM->SBUF) + DVE spike (is_ge) + DVE masked-reset carry.
The fp8 layers use per-row scales, so their carry is added on Pool instead.
S*D products are split across DVE (direct from PSUM), ACT-evict + fp16 DVE
(2x mode), and ACT-evict + Pool, to balance all four engines in retention.
"""

from contextlib import ExitStack

import numpy as np
import ml_dtypes

import concourse.bacc as bacc
import concourse.tile as tile
from concourse import mybir
from concourse.bass_utils import run_bass_kernel_spmd

f32 = mybir.dt.float32
f32r = mybir.dt.float32r
fp16 = mybir.dt.float16
fp8 = mybir.dt.float8e4
Alu = mybir.AluOpType
Act = mybir.ActivationFunctionType
DR = mybir.MatmulPerfMode.DoubleRow

T, B, N, C = 4, 8, 512, 512
HID = 2048
H = 8
EPS = 1e-5
NT = N // 128
CT = C // 128
HT = HID // 128

_CACHE = {}


def _build():
    nc = bacc.Bacc("TRN2", target_bir_lowering=False, debug=False)

    xbr = nc.declare_dram_parameter("xbr", [T, CT, 128, N], f32r, isOutput=False)
    xb = nc.declare_dram_parameter("xb", [T, CT, 128, N], f32, isOutput=False)
    ident16_e = nc.declare_dram_parameter("ident16", [128, 128], fp16, isOutput=False)
    # diag(2^-(t-1)) for folding the LIF carry into PSUM, t = 2..4
    idc_e = nc.declare_dram_parameter("idc", [3, 128, 128], f32r, isOutput=False)
    w_in = {}
    for nm, ci, co in [("qw", C, C), ("kw", C, C), ("vw", C, C), ("w1", C, HID)]:
        w_in[nm] = nc.declare_dram_parameter(nm + "r", [ci, co], f32r, isOutput=False)
    for nm, ci, co in [("pw", C, C), ("w2", HID, C)]:
        for j in range(2):
            w_in[f"{nm}{j}"] = nc.declare_dram_parameter(f"{nm}{j}", [ci, co], fp8,
                                                         isOutput=False)
    consts_e = nc.declare_dram_parameter("consts", [128, T, 44], f32, isOutput=False)
    dmat_e = nc.declare_dram_parameter("dmat", [H, NT, 128, N], fp16, isOutput=False)
    out_e = nc.declare_dram_parameter("out", [T, CT, 128, N], f32, isOutput=True)

    with tile.TileContext(nc) as tc, ExitStack() as ctx:
        pers = ctx.enter_context(tc.tile_pool(name="pers", bufs=1))
        work = ctx.enter_context(tc.tile_pool(name="work", bufs=1))

        ident16 = pers.tile([128, 128], fp16, name="ident16")
        idc = [pers.tile([128, 128], f32r, name=f"idc{t}") for t in (2, 3, 4)]
        bias_sb = {}
        sc_sb = {}

        def carry_mm(ps, t, carry):
            """Fold carry*2^-(t-1) into a PSUM accumulation group (opens it)."""
            nc.tensor.matmul(ps[:], idc[t - 2][:], carry[:], start=True, stop=False)

        def lif_fold(t, ps, bias_ap, theta, spike_ap, carry_tag, cpool):
            """Carry already folded in PSUM. A = 2^(t-1)*ps + bias (ACT);
            spike (DVE); carry' = A*(A<thr) in f32r (DVE)."""
            sc = float(2.0 ** (t - 1))
            A = work.tile([128, N], f32, name="lifA", tag="lifA", bufs=6)
            if bias_ap is None:
                nc.scalar.activation(A[:], ps[:], Act.Copy, bias=0.0, scale=sc)
            else:
                nc.scalar.activation(A[:], ps[:], Act.Identity, bias=bias_ap, scale=sc)
            thr = float(theta * (2.0 ** t))
            nc.vector.tensor_scalar(spike_ap, A[:], thr, None, Alu.is_ge)
            if carry_tag is not None:
                Cn = cpool.tile([128, N], f32r, name="lifC", tag=carry_tag, bufs=1)
                nc.vector.scalar_tensor_tensor(Cn[:], A[:], thr, A[:], Alu.is_lt, Alu.mult)
                return Cn
            return None

        def lif_pool(t, ps, bias_ap, scale_ap, carry_in, spike_ap, carry_tag, cpool):
            """fp8-layer path (per-row scale): zb = scale*ps + bias (ACT);
            A = zb + carry (Pool); spike / carry (DVE)."""
            zb = work.tile([128, N], f32, name="lifzb", tag="lifzb", bufs=4)
            nc.scalar.activation(zb[:], ps[:], Act.Identity, bias=bias_ap, scale=scale_ap)
            if carry_in is None:
                A = zb
            else:
                A = work.tile([128, N], f32, name="lifA", tag="lifA", bufs=6)
                nc.gpsimd.tensor_tensor(A[:], carry_in[:], zb[:], Alu.add)
            thr = float(2.0 ** t)
            nc.vector.tensor_scalar(spike_ap, A[:], thr, None, Alu.is_ge)
            if carry_tag is not None:
                Cn = cpool.tile([128, N], f32, name="lifCp", tag=carry_tag, bufs=1)
                nc.vector.scalar_tensor_tensor(Cn[:], A[:], thr, A[:], Alu.is_lt, Alu.mult)
                return Cn
            return None

        with tc.tile_pool(name="pw8_pool", bufs=1) as pw8_pool, \
             tc.tile_pool(name="spk_o_pool", bufs=1) as spk_o_pool, \
             tc.tile_pool(name="wmlp_pool", bufs=1) as wmlp_pool:
            # o spikes: fp8 pair tiles [128, 2, N]; j-th tile holds head-pairs 2j, 2j+1
            os8 = {}
            for t in range(T):
                for j in range(2):
                    os8[t, j] = spk_o_pool.tile([128, 2, N], fp8, name=f"os8_{t}_{j}")

            with tc.tile_pool(name="qkvn_pool", bufs=1) as qkvn_pool:
                qs = {}
                ks = {}
                vn = {}
                with tc.tile_pool(name="wqkv_pool", bufs=1) as wqkv_pool, \
                     tc.tile_pool(name="xr_pool", bufs=1) as xr_pool, \
                     tc.tile_pool(name="vsT_pool", bufs=1) as vsT_pool, \
                     tc.tile_pool(name="psA", bufs=1, space="PSUM") as psA:
                    # v weights on the second (gpsimd) DMA queue, x on the
                    # sync queue, so wave-1 inputs stream in parallel.
                    wq = {}
                    srcs = {}
                    for nm in ("vw", "qw", "kw"):
                        wq[nm] = wqkv_pool.tile([128, CT, C], f32r, name=f"w_{nm}")
                        srcs[nm] = w_in[nm].rearrange("(kt p) o -> p kt o", p=128)
                    for kt in range(CT):
                        for nm in ("vw", "qw"):
                            nc.scalar.dma_start(wq[nm][:, kt, :], srcs[nm][:, kt, :])
                    for kt in range(CT):
                        nc.scalar.dma_start(wq["kw"][:, kt, :], srcs["kw"][:, kt, :])
                    # packed constants: one DMA, before x (biases gate LIF)
                    consts = pers.tile([128, T, 44], f32, name="consts")
                    nc.sync.dma_start(consts[:], consts_e[:, :, :])
                    off = 0
                    for nm, cnt in [("qb", CT), ("kb", CT), ("vb", CT), ("pb", CT),
                                    ("b1", HT), ("b2", CT)]:
                        bias_sb[nm] = (consts, off)
                        off += cnt
                    for nm in ("pA", "fA"):
                        sc_sb[nm] = (consts, off)
                        off += CT

                    def bap(nm, t, ot):
                        tl, o = bias_sb[nm]
                        return tl[:, t - 1, o + ot:o + ot + 1]

                    def sap(nm, t, ot):
                        tl, o = sc_sb[nm]
                        return tl[:, t - 1, o + ot:o + ot + 1]
                    bias_sb["ap"] = bap
                    sc_sb["ap"] = sap
                    # x: pre-rounded on host, lands straight in f32r tiles;
                    # ident/idc go on the queue after the first wave's inputs
                    xr = {}
                    for t in range(T):
                        for ct in range(CT):
                            xrt = xr_pool.tile([128, N], f32r, name="xr", tag="xr", bufs=16)
                            nc.sync.dma_start(xrt[:], xbr[t, ct])
                            xr[t, ct] = xrt
                        if t == 0:
                            nc.sync.dma_start(ident16[:], ident16_e[:, :])
                            for _i in range(3):
                                nc.sync.dma_start(idc[_i][:], idc_e[_i])

                    # ---- stage QKV: t-major waves of 12 chains, carry folded
                    # into PSUM; v transpose after each wave.
                    vsT = {}
                    carries = {}
                    for t in range(1, T + 1):
                        for nm, bnm in [("vw", "vb"), ("qw", "qb"), ("kw", "kb")]:
                            for ot in range(CT):
                                ps = psA.tile([128, N], f32, name="psq", tag="psq", bufs=6)
                                carry = carries.get((nm, ot))
                                if carry is not None:
                                    carry_mm(ps, t, carry)
                                for kt in range(CT):
                                    nc.tensor.matmul(ps[:],
                                                     wq[nm][:, kt, ot * 128:(ot + 1) * 128],
                                                     xr[t - 1, kt][:],
                                                     start=(kt == 0 and carry is None),
                                                     stop=(kt == CT - 1))
                                if nm == "vw":
                                    st = vsT_pool.tile([128, N], fp16, name="spk_v",
                                                       tag="spk_v", bufs=8)
                                    vsT[t - 1, ot] = st
                                else:
                                    st = qkvn_pool.tile([128, N], fp8, name=f"spk_{nm}",
                                                        tag=f"spk_{nm}", bufs=16)
                                    if nm == "qw":
                                        qs[t - 1, ot] = st
                                    else:
                                        ks[t - 1, ot] = st
                                carries[nm, ot] = lif_fold(
                                    t, ps, bias_sb["ap"](bnm, t, ot), 1.0,
                                    st[:], f"c_{nm}_{ot}" if t < T else None, xr_pool)
                        # v spikes of this wave -> natural layout (fp16)
                        for nt in range(NT):
                            ps = psA.tile([128, C], f32, name="psv", tag="psx", bufs=2)
                            for ct in range(CT):
                                nc.tensor.matmul(
                                    ps[:, ct * 128:(ct + 1) * 128],
                                    vsT[t - 1, ct][:, nt * 128:(nt + 1) * 128],
                                    ident16[:], start=True, stop=True)
                            vt = qkvn_pool.tile([128, C], fp16, name="vn", tag="vn",
                                                bufs=16)
                            nc.scalar.copy(vt[:], ps[:])
                            vn[t - 1, nt] = vt

                # ---- stage RET (retention): head pairs; out-matmuls run one
                # timestep behind the scores so S*D (on Pool) overlaps the PE.
                with tc.tile_pool(name="dpool", bufs=2) as dpool, \
                     tc.tile_pool(name="spool", bufs=1) as spool, \
                     tc.tile_pool(name="psR", bufs=1, space="PSUM") as psR:
                    # proj + MLP weights loaded here (second DMA queue) so
                    # they land while retention computes.
                    pw8 = {}
                    for j in range(2):
                        wt = pw8_pool.tile([128, CT, C], fp8, name=f"w_pw{j}")
                        nc.scalar.dma_start(
                            wt[:], w_in[f"pw{j}"].rearrange("(kt p) o -> p kt o", p=128))
                        pw8[j] = wt
                    w1r = wmlp_pool.tile([128, CT, HID], f32r, name="w_w1r")
                    nc.scalar.dma_start(
                        w1r[:], w_in["w1"].rearrange("(kt p) o -> p kt o", p=128))
                    w28 = {}
                    for j in range(2):
                        wt = wmlp_pool.tile([128, HT, C], fp8, name=f"w_w2{j}")
                        nc.scalar.dma_start(
                            wt[:], w_in[f"w2{j}"].rearrange("(kt p) o -> p kt o", p=128))
                        w28[j] = wt

                    for hp in range(H // 2):
                        h0, h1 = 2 * hp, 2 * hp + 1
                        dm0 = dpool.tile([128, NT, N], fp16, name="dm0", tag="dm0")
                        nc.sync.dma_start(dm0[:], dmat_e[h0].rearrange("mt p n -> p mt n"))
                        dm1 = dpool.tile([128, NT, N], fp16, name="dm1", tag="dm1")
                        nc.sync.dma_start(dm1[:], dmat_e[h1].rearrange("mt p n -> p mt n"))
                        carry = None
                        stage = []  # (t, stiles) awaiting out-matmul
                        for t in range(T):
                            stiles = []
                            for mt in range(NT):
                                ps0 = psR.tile([128, N], f32, name="ps_s0", tag="ps_s0",
                                               bufs=3)
                                nc.tensor.matmul(ps0[:],
                                                 ks[t, hp][0:64, mt * 128:(mt + 1) * 128],
                                                 qs[t, hp][0:64, :], start=True, stop=True)
                                ps1 = psR.tile([128, N], f32, name="ps_s1", tag="ps_s1",
                                               bufs=3)
                                nc.tensor.matmul(ps1[:],
                                                 ks[t, hp][64:128, mt * 128:(mt + 1) * 128],
                                                 qs[t, hp][64:128, :], start=True, stop=True)
                                s0 = spool.tile([128, N], fp16, name="sd0", tag="sd0",
                                                bufs=8)
                                s1 = spool.tile([128, N], fp16, name="sd1", tag="sd1",
                                                bufs=8)
                                # S*D split ~5:3 between DVE and Pool; the
                                # DVE share goes via an ACT fp16 eviction so
                                # the DVE multiply runs in 2x fp16 mode.
                                def sd(dst, psx, dm_ap, mode):
                                    # GPSIMD cannot read PSUM: Pool work gets
                                    # an ACT fp16 eviction first.
                                    if mode == 2:  # direct DVE from PSUM
                                        nc.vector.tensor_tensor(dst[:], psx[:], dm_ap,
                                                                Alu.mult)
                                        return
                                    sc = spool.tile([128, N], fp16, name="s16",
                                                    tag="s16", bufs=5)
                                    nc.scalar.copy(sc[:], psx[:])
                                    if mode == 1:  # fast fp16 DVE
                                        nc.vector.tensor_tensor(dst[:], sc[:], dm_ap,
                                                                Alu.mult)
                                    else:  # Pool, SBUF-only
                                        nc.gpsimd.tensor_tensor(dst[:], sc[:], dm_ap,
                                                                Alu.mult)
                                m0, m1 = [(2, 2), (2, 1), (1, 1), (0, 0)][mt]
                                sd(s0, ps0, dm0[:, mt, :], m0)
                                sd(s1, ps1, dm1[:, mt, :], m1)
                                stiles.append((s0, s1))
                            stage.append((t, stiles))
                            if len(stage) == 2 or t == T - 1:
                                for tt_, stl in stage if t == T - 1 else stage[:1]:
                                    pso = psR.tile([128, N], f32, name="ps_o", tag="ps_o",
                                                   bufs=2)
                                    if carry is not None:
                                        carry_mm(pso, tt_ + 1, carry)
                                    for mt in range(NT):
                                        s0, s1 = stl[mt]
                                        nc.tensor.matmul(
                                            pso[0:64, :],
                                            vn[tt_, mt][:, h0 * 64:(h0 + 1) * 64], s0[:],
                                            start=(mt == 0 and carry is None), stop=False)
                                        nc.tensor.matmul(
                                            pso[64:128, :],
                                            vn[tt_, mt][:, h1 * 64:(h1 + 1) * 64], s1[:],
                                            start=(mt == 0 and carry is None),
                                            stop=(mt == NT - 1))
                                    carry = lif_fold(
                                        tt_ + 1, pso, None, 0.5,
                                        os8[tt_, hp // 2][:, hp % 2, :],
                                        "c_o" if tt_ < T - 1 else None, spool)
                                stage = stage[1:] if t < T - 1 else []

            # ---- merged PROJ + MLP, t-outer; fc2 runs one timestep behind.
            with tc.tile_pool(name="mwork", bufs=1) as mwork, \
                 tc.tile_pool(name="xtin_pool", bufs=1) as xtin_pool, \
                 tc.tile_pool(name="psM", bufs=1, space="PSUM") as psM:
                cp = {}
                c1 = {}
                c2 = {}
                x2 = {}
                h8 = {}
                fc2_pend = []

                def emit_fc2(t):
                    # fc2 fp8-DR 2-term for timestep t (1-based)
                    for ot in range(CT):
                        psa = psM.tile([128, N], f32, name="psf2A", tag="psf2A", bufs=2)
                        for j in range(HT // 2):
                            for trm in (0, 1):
                                nc.tensor.matmul(
                                    psa[:],
                                    w28[trm][:, 2 * j:2 * j + 2, ot * 128:(ot + 1) * 128],
                                    h8[t, j][:, :, :],
                                    start=(j == 0 and trm == 0),
                                    stop=(j == HT // 2 - 1 and trm == 1),
                                    perf_mode=DR)
                        stm = mwork.tile([128, N], fp16, name="spk_m", tag="spk_m", bufs=2)
                        c2[ot] = lif_pool(t, psa, bias_sb["ap"]("b2", t, ot),
                                          sc_sb["ap"]("fA", t, ot),
                                          c2.get(ot), stm[:],
                                          f"c2_{ot}" if t < T else None, mwork)
                        ot_t = mwork.tile([128, N], f32, name="outT", tag="outT", bufs=3)
                        if ot % 2 == 0:
                            nc.gpsimd.tensor_tensor(ot_t[:], x2[t, ot][:], stm[:], Alu.add)
                        else:
                            nc.vector.tensor_tensor(ot_t[:], x2[t, ot][:], stm[:], Alu.add)
                        nc.sync.dma_start(out_e[t - 1, ot], ot_t[:])

                for t in range(1, T + 1):
                    # proj fp8-DR 2-term + LIF -> attn spikes; x2 = x + attn
                    for ot in range(CT):
                        psa = psM.tile([128, N], f32, name="pspA", tag="pspA", bufs=2)
                        for j in range(CT // 2):
                            for trm in (0, 1):
                                nc.tensor.matmul(
                                    psa[:],
                                    pw8[trm][:, 2 * j:2 * j + 2, ot * 128:(ot + 1) * 128],
                                    os8[t - 1, j][:, :, :],
                                    start=(j == 0 and trm == 0),
                                    stop=(j == CT // 2 - 1 and trm == 1),
                                    perf_mode=DR)
                        stp = mwork.tile([128, N], fp16, name="spk_p", tag="spk_p", bufs=4)
                        cp[ot] = lif_pool(t, psa, bias_sb["ap"]("pb", t, ot),
                                          sc_sb["ap"]("pA", t, ot),
                                          cp.get(ot), stp[:],
                                          f"cp_{ot}" if t < T else None, mwork)
                        xt = xtin_pool.tile([128, N], f32, name="xtin", tag="xtin", bufs=3)
                        nc.sync.dma_start(xt[:], xb[t - 1, ot])
                        # one DVE add producing the f32r x2 used by both fc1
                        # and the residual output add
                        x2r = mwork.tile([128, N], f32r, name="x2r", tag="x2r", bufs=8)
                        nc.vector.tensor_tensor(x2r[:], xt[:], stp[:], Alu.add)
                        x2[t, ot] = x2r
                        x2[t, ot, "r"] = x2r

                    # fc2 of previous timestep fills the PE while fc1 LIF drains
                    if fc2_pend:
                        emit_fc2(fc2_pend.pop())

                    # fc1 f32r single pass + LIF -> h spikes (fp8 pair tiles)
                    for ot in range(HT):
                        ps = psM.tile([128, N], f32, name="psf1", tag="psf1", bufs=4)
                        carry = c1.get(ot)
                        if carry is not None:
                            carry_mm(ps, t, carry)
                        for kt in range(CT):
                            nc.tensor.matmul(ps[:], w1r[:, kt, ot * 128:(ot + 1) * 128],
                                             x2[t, kt, "r"][:],
                                             start=(kt == 0 and carry is None),
                                             stop=(kt == CT - 1))
                        if ot % 2 == 0:
                            h8[t, ot // 2] = mwork.tile([128, 2, N], fp8, name="spk_h",
                                                        tag="spk_h", bufs=16)
                        c1[ot] = lif_fold(t, ps, bias_sb["ap"]("b1", t, ot), 1.0,
                                          h8[t, ot // 2][:, ot % 2, :],
                                          f"c1_{ot}" if t < T else None, mwork)
                    fc2_pend.append(t)

                emit_fc2(fc2_pend.pop())

    nc.finalize()
    return nc


def _round12(x):
    x = np.ascontiguousarray(x, np.float32)
    m = x.view(np.uint32)
    lsb = (m >> 12) & 1
    m = (m + 0x7FF + lsb) & np.uint32(0xFFFFF000)
    return m.view(np.float32)


def _host_prep(inputs):
    def fold(w, b, bn):
        g, bb, m, v = [bn[i].astype(np.float64) for i in range(4)]
        A = g / np.sqrt(v + EPS)
        W = w.astype(np.float64) * A[:, None]
        bias = (b.astype(np.float64) - m) * A + bb
        return W, bias

    def bias_layout(bias):
        co = bias.shape[0]
        arr = np.stack([(bias * (2.0 ** t)).reshape(co // 128, 128).T
                        for t in range(T)], axis=1)
        return np.ascontiguousarray(arr.astype(np.float32))

    def split8(W):
        """Per-row 2-term e4m3 split, single pow2 scale per row."""
        e4 = ml_dtypes.float8_e4m3
        rmaxA = np.abs(W).max(axis=1)
        sA = 2.0 ** np.floor(np.log2(208.0 / np.maximum(rmaxA, 1e-30)))
        Ws = W * sA[:, None]
        q0 = Ws.astype(e4)
        e0 = Ws - q0.astype(np.float64)
        q1 = e0.astype(e4)
        for q in (q0, q1):
            assert np.isfinite(q.astype(np.float64)).all()
        return (q0, q1), sA

    def scale_layout(s):
        co = s.shape[0]
        arr = np.stack([(s * (2.0 ** t)).reshape(co // 128, 128).T
                        for t in range(T)], axis=1)
        return np.ascontiguousarray(arr.astype(np.float32))

    feed = {}
    packed = {}
    for wkey, bkey, bnkey, bout in [("qw", "qb", "qbn", "qb"), ("kw", "kb", "kbn", "kb"),
                                    ("vw", "vb", "vbn", "vb"), ("w1", "b1", "bn1", "b1")]:
        W, bias = fold(inputs[wkey], inputs[bkey], inputs[bnkey])
        feed[wkey + "r"] = _round12(np.ascontiguousarray(W.T, np.float32))
        packed[bout] = bias_layout(bias)
    for wkey, bkey, bnkey, bout, sa in [("pw", "pb", "pbn", "pb", "pA"),
                                        ("w2", "b2", "bn2", "b2", "fA")]:
        W, bias = fold(inputs[wkey], inputs[bkey], inputs[bnkey])
        (q0, q1), sA = split8(W)
        for j, q in enumerate((q0, q1)):
            feed[f"{wkey}{j}"] = np.ascontiguousarray(q.T).view(np.uint8)
        packed[bout] = bias_layout(bias)
        packed["sc" + sa] = scale_layout(1.0 / sA)
    feed["consts"] = np.ascontiguousarray(np.concatenate(
        [packed[k] for k in ("qb", "kb", "vb", "pb", "b1", "b2", "scpA", "scfA")],
        axis=2))

    gamma = 1.0 - 2.0 ** (-5.0 - np.arange(H, dtype=np.float64))
    idx = np.arange(N, dtype=np.float64)
    dist = np.abs(idx[:, None] - idx[None, :])
    scale = (C // H) ** -0.5
    dm = np.empty((H, NT, 128, N), np.float16)
    for h in range(H):
        dm[h] = ((gamma[h] ** dist) * scale).reshape(NT, 128, N).astype(np.float16)
    feed["dmat"] = dm
    feed["ident16"] = np.eye(128, dtype=np.float16)
    idc = np.zeros((3, 128, 128), np.float32)
    for i, t in enumerate(range(2, T + 1)):
        idc[i] = np.eye(128, dtype=np.float32) * (2.0 ** -(t - 1))
    feed["idc"] = idc
    return feed


def kernel(**inputs):
    if "nc" not in _CACHE:
        _CACHE["nc"] = _build()
    nc = _CACHE["nc"]
    feed = _host_prep(inputs)
    x = inputs["x"]
    in_maps = []
    for b in range(B):
        m = dict(feed)
        xt = np.ascontiguousarray(x[:, b].transpose(0, 2, 1).reshape(T, CT, 128, N))
        m["xb"] = xt
        m["xbr"] = _round12(xt)
        in_maps.append(m)
    res = None
    last_err = None
    for _attempt in range(3):
        try:
            res = run_bass_kernel_spmd(nc, in_maps, list(range(B)))
            break
        except Exception as e:  # transient NRT device wedges recover on retry
            last_err = e
    if res is None:
        raise last_err
    out = np.empty((T, B, N, C), np.float32)
    for b in range(B):
        oT = res.results[b]["out"].reshape(T, C, N)
        out[:, b] = oT.transpose(0, 2, 1)
    return out
